# revision 39
# baseline (speedup 1.0000x reference)
"""AttentiveTransformer (Dense + BN(inference) + prior-scale + sparsemax) on 8 trn2 cores.

Math (per reference):
    z   = (x @ W + b) * inv + (beta - mm*inv),  inv = gamma/sqrt(mv+eps)
    z   = z * prior_scales
    out = sparsemax(z)  (rowwise simplex projection)

Strategy (v7):
  - Host folds BN into W/bias; W and x ship as fp16 (the PE moving-stream
    is 2 B/cycle/partition, so fp16 halves GEMM time vs f32; fp8 DoubleRow
    was measured to give 2x MACs/byte but the same bytes/cycle, so the
    2-level-accuracy fp8 split loses to fp16).
  - Data-parallel over batch: 8192 rows -> 8 cores x 8 row-tiles of 128.
  - GEMM in 4 phases.  Phase A runs tiles 0-3 x W-columns 0:1024 K-MAJOR
    (k outer, tile inner, 4 psum pairs = all 8 banks): per K-chunk the PE
    does 4 tiles' worth of work, so W is consumed at ~140 GB/s, matching
    the HBM stream -- the PE starts as soon as the first x K-slab and
    first W chunk land (~11 us) and never waits on the W stream.  Phases
    B/C/D are tile-major over the now-resident W.  x ships as K-slabs
    (first three split in half) to unblock phase A's wavefront.
  - Exact sparsemax threshold from top-16 candidates: support here is
    <= 10 per row with <= 7 per 1024-column half, so DVE MAX8 (top-8
    sorted per partition) over each half provably contains the support.
    Sort 16 candidates (max8 + match_replace + max8), then
    tau* = max_k (cumsum_k - 1)/k -- exact, no Newton scans.
  - Final out = relu(z - tau) split ScalarE/DVE by column half, each half
    DMA'd on its own queue (Act / GpSimd) as soon as it's done.
  - Measured rel err vs the f32 reference: 1.9e-3 (gate 2e-2).  Host-side
    rowsum check falls back to a 32-candidate (quarter-wise) variant if
    sparsemax support ever exceeds 8 in one half.
"""

import sys

for _p in ("/opt/trn_rl_repo",):
    if _p not in sys.path:
        sys.path.append(_p)

from contextlib import ExitStack

import numpy as np

import concourse.bacc as bacc
import concourse.bass as bass
import concourse.mybir as mybir
import concourse.tile as tile
from concourse import bass_utils

B, F = 8192, 2048
NCORES = 8
BS = B // NCORES          # rows per core
P = 128                   # SBUF partitions
KC = F // P               # contraction chunks (16)
PW = 1024                 # psum pair tile width
MT = BS // P              # row tiles per core (8)
NSPLIT = 3                # leading x K-slabs split in half for phase A
BN_EPS = 1e-3
DEFAULT_ITERS = 2         # unused in v7; kept for test.py compat

f32 = mybir.dt.float32
f16 = mybir.dt.float16


def build_program(with_prior: bool, nseg: int = 2):
    """Per-core Bass program (SPMD).  nseg: column segments for the top-8
    candidate extraction (2 -> 16 candidates, 4 -> 32)."""
    nc = bacc.Bacc()
    ncand = 8 * nseg
    xs = nc.dram_tensor("xs", [KC, P, BS], f16, kind="ExternalInput")
    wprep = nc.dram_tensor("wprep", [2, P, KC, PW], f16, kind="ExternalInput")
    bprep = nc.dram_tensor("bprep", [P, F + ncand], f32, kind="ExternalInput")
    rkg = None
    if nseg == 2:
        # -1/(i+j) grid for the prefix-pair tau identity (entry (0,0) = -1.0,
        # a harmless dummy whose tk value +1 can never win the min)
        rkg = nc.dram_tensor("rkg", [P, 9, 9], f32, kind="ExternalInput")
    prior = None
    if with_prior:
        prior = nc.dram_tensor("prior", [BS, F], f32, kind="ExternalInput")
    out = nc.dram_tensor("out", [BS, F], f16, kind="ExternalOutput")

    relu = mybir.ActivationFunctionType.Relu
    AO = mybir.AluOpType
    SEGW = F // nseg

    with tile.TileContext(nc) as tc, ExitStack() as ctx:
        consts = ctx.enter_context(tc.tile_pool(name="consts", bufs=1))
        wpool = ctx.enter_context(tc.tile_pool(name="w", bufs=1))
        xpool = ctx.enter_context(tc.tile_pool(name="x", bufs=1))
        zpool = ctx.enter_context(tc.tile_pool(name="z", bufs=1))
        opool = ctx.enter_context(tc.tile_pool(name="o", bufs=3))
        vpool = ctx.enter_context(tc.tile_pool(name="vec", bufs=1))
        spool = ctx.enter_context(tc.tile_pool(name="scr", bufs=2))
        psum = ctx.enter_context(tc.tile_pool(name="psum", bufs=4, space="PSUM"))
        prpool = None
        if with_prior:
            prpool = ctx.enter_context(tc.tile_pool(name="pr", bufs=3))

        # ---- x K-slabs on the Act queue; first NSPLIT slabs split a/b ----
        xa_t = [None] * KC   # tiles 0-3 part (cols 0:512 of the slab)
        xb_t = [None] * KC   # tiles 4-7 part
        xf_t = [None] * KC   # full slab
        for k in range(NSPLIT):
            xa_t[k] = xpool.tile([P, BS // 2], f16, tag=f"xa{k}", name=f"xa{k}")
            nc.scalar.dma_start(out=xa_t[k], in_=xs[k][:, 0:BS // 2])

        # ---- Sync queue: W cp0 per-K tiles (k-paced for phase A), then the
        # small xb halves (needed at phase B ~40us, land ~29us), then the
        # W cp1 bulk (needed ~70us).  Keeping at most two streams active at
        # once matters: a third concurrent stream backpressures SBUF writes
        # and stretches every PE instruction ~18%.
        w0_t = [None] * KC
        for k in range(KC):
            w0_t[k] = wpool.tile([P, PW], f16, tag=f"w0_{k}", name=f"w0_{k}")
            nc.sync.dma_start(out=w0_t[k], in_=wprep[0][:, k])
        for j in range(NSPLIT):
            xb_t[j] = xpool.tile([P, BS // 2], f16, tag=f"xb{j}", name=f"xb{j}")
            nc.sync.dma_start(out=xb_t[j], in_=xs[j][:, BS // 2:])
        w1_t = wpool.tile([P, KC, PW], f16, tag="w1")
        nc.sync.dma_start(out=w1_t, in_=wprep[1])

        # Act queue: x slabs are consumed at ~140 GB/s by phase A while the
        # queue delivers ~190 GB/s; the accumulated slack fits bias-lo after
        # xf8 (needed at A's drains ~39us).
        bp_t = consts.tile([P, F + ncand], f32)
        for k in range(NSPLIT, KC):
            xf_t[k] = xpool.tile([P, BS], f16, tag=f"xf{k}", name=f"xf{k}")
            nc.scalar.dma_start(out=xf_t[k], in_=xs[k])
            if k == 8:
                nc.scalar.dma_start(out=bp_t[:, 0:PW], in_=bprep[:, 0:PW])
        nc.scalar.dma_start(out=bp_t[:, PW:], in_=bprep[:, PW:])
        rk_t = bp_t[:, F:F + ncand]

        # warm the PE clock (p-state ramps over ~3us) on throwaway matmuls
        # while the first x/W DMAs are still in flight
        warm = consts.tile([P, 512], f16)
        nc.gpsimd.memset(warm, 0.0)
        ps_w = psum.tile([P, PW], f32, tag="ps", name="ps_warm")
        for _ in range(10):
            nc.tensor.matmul(ps_w[:, 0:512], warm[:, 0:P], warm,
                             start=True, stop=True)

        def x_ap(k, m):
            if xf_t[k] is not None:
                return xf_t[k][:, m * P:(m + 1) * P]
            if m < 4:
                return xa_t[k][:, m * P:(m + 1) * P]
            return xb_t[k][:, (m - 4) * P:(m - 3) * P]

        z_t = [zpool.tile([P, F], f16, tag=f"z{m}", name=f"z{m}")
               for m in range(MT)]
        cand_t = [vpool.tile([P, ncand], f16, tag=f"cand{m}", name=f"c{m}")
                  for m in range(MT)]
        csA_t = csB_t = rkg_t = None
        if nseg == 2:
            # zero-prefixed cumsum vectors for the tau prefix-pair grid:
            # csA over the cp0 top-8, csB over the cp1 top-8
            csA_t = [vpool.tile([P, 9, 1], f32, tag=f"csA{m}", name=f"a{m}")
                     for m in range(MT)]
            csB_t = [vpool.tile([P, 1, 9], f32, tag=f"csB{m}", name=f"b{m}")
                     for m in range(MT)]
            for m in range(MT):
                nc.gpsimd.memset(csA_t[m][:, 0:1, 0], 0.0)
                nc.gpsimd.memset(csB_t[m][:, 0, 0:1], 0.0)
            rkg_t = consts.tile([P, 9, 9], f32)
            nc.scalar.dma_start(out=rkg_t, in_=rkg[:, :, :])

        def drain_and_max(m, cp, ps):
            s = slice(cp * PW, (cp + 1) * PW)
            if with_prior:
                prt = prpool.tile([P, PW], f32, tag="pr", name=f"pr{m}_{cp}")
                nc.scalar.dma_start(out=prt, in_=prior[m * P:(m + 1) * P, s])
                tmp = spool.tile([P, PW], f32, tag="tmp", name=f"tp{m}_{cp}")
                nc.vector.tensor_tensor(tmp, ps, bp_t[:, s], op=AO.add)
                nc.vector.tensor_tensor(z_t[m][:, s], tmp, prt, op=AO.mult)
            else:
                nc.vector.tensor_tensor(z_t[m][:, s], ps, bp_t[:, s], op=AO.add)
            for g in range(nseg // 2):
                seg = cp * (nseg // 2) + g
                nc.vector.max(cand_t[m][:, seg * 8:(seg + 1) * 8],
                              z_t[m][:, seg * SEGW:(seg + 1) * SEGW])
            if nseg == 2 and cp == 0:
                # cp0 cumsum runs here, long before the tail needs it
                nc.vector.tensor_tensor_scan(
                    csA_t[m][:, 1:9, 0], cand_t[m][:, 0:8], cand_t[m][:, 0:8],
                    0.0, op0=AO.add, op1=AO.bypass)

        def w_ap(k, cp, hs):
            if cp == 0:
                return w0_t[k][:, hs]
            return w1_t[:, k, hs]

        def gemm_block(m, cp):
            # tile-major block: h outer, k inner (baseline-proven order).
            # For the very last block the h0 half-drain runs during the h1
            # matmuls, shortening the end-of-kernel critical chain ~0.6us.
            split = cp == 1 and m == MT - 1 and not with_prior
            ps = psum.tile([P, PW], f32, tag="ps", name=f"ps{m}_{cp}")
            for h in range(2):
                hs = slice(h * 512, (h + 1) * 512)
                for k in range(KC):
                    nc.tensor.matmul(
                        ps[:, hs], x_ap(k, m), w_ap(k, cp, hs),
                        start=(k == 0), stop=(k == KC - 1))
                if split and h == 0:
                    nc.vector.tensor_tensor(z_t[m][:, PW:PW + 512],
                                            ps[:, 0:512],
                                            bp_t[:, PW:PW + 512], op=AO.add)
            if split:
                nc.vector.tensor_tensor(z_t[m][:, PW + 512:F], ps[:, 512:PW],
                                        bp_t[:, PW + 512:F], op=AO.add)
                nc.vector.max(cand_t[m][:, 8:16], z_t[m][:, PW:F])
            else:
                drain_and_max(m, cp, ps)

        def tail(m):
            cand = cand_t[m]
            nt = vpool.tile([P, 1], f32, tag=f"nt{m}", name=f"n{m}")
            if nseg == 2:
                # prefix-pair grid: tau* = max_{i,j} (csA_i + csB_j - 1)/(i+j)
                # (top-k of A u B is always prefix(A) u prefix(B)); rkg holds
                # -1/(i+j) so the min-accumulate yields nt = -tau* directly
                nc.vector.tensor_tensor_scan(
                    csB_t[m][:, 0, 1:9], cand[:, 8:16], cand[:, 8:16],
                    0.0, op0=AO.add, op1=AO.bypass)
                a_g, b_g = bass.broadcast_tensor_aps(csA_t[m][:, :, :],
                                                     csB_t[m][:, :, :])
                gt = vpool.tile([P, 9, 9], f32, tag=f"g{m}", name=f"g{m}")
                nc.vector.tensor_tensor(gt, a_g, b_g, op=AO.add)
                tk = vpool.tile([P, 9, 9], f32, tag=f"tk{m}", name=f"t{m}")
                nc.vector.scalar_tensor_tensor(tk, gt, -1.0, rkg_t,
                                               op0=AO.add, op1=AO.mult)
                dmy = spool.tile([P, 81], f32, tag="dmy", name=f"d{m}")
                nc.vector.tensor_scalar(dmy, tk, 0.0, None,
                                        op0=AO.add, op1=AO.min, accum_out=nt)
            else:
                srt = vpool.tile([P, ncand], f16, tag=f"srt{m}", name=f"s{m}")
                nc.vector.max(srt[:, 0:8], cand)
                prev = cand
                for r in range(1, nseg):
                    mr = vpool.tile([P, ncand], f16, tag=f"mr{m}_{r}",
                                    name=f"m{m}_{r}")
                    nc.vector.match_replace(mr, srt[:, (r - 1) * 8:r * 8],
                                            prev, -60000.0)
                    nc.vector.max(srt[:, r * 8:(r + 1) * 8], mr)
                    prev = mr
                # rk holds NEGATED 1/k, so tk = (cs-1)*(-1/k) = -t_k and the
                # min-accumulate yields nt = -tau* directly
                cs = vpool.tile([P, ncand], f32, tag=f"cs{m}", name=f"q{m}")
                nc.vector.tensor_tensor_scan(cs, srt, srt, 0.0,
                                             op0=AO.add, op1=AO.bypass)
                tk = vpool.tile([P, ncand], f32, tag=f"tk{m}", name=f"t{m}")
                nc.vector.scalar_tensor_tensor(tk, cs, -1.0, rk_t,
                                               op0=AO.add, op1=AO.mult)
                dmy = spool.tile([P, ncand], f32, tag="dmy", name=f"d{m}")
                nc.vector.tensor_scalar(dmy, tk, 0.0, None,
                                        op0=AO.add, op1=AO.min, accum_out=nt)
            # out = relu(z - tau).  The DVE runs fp16 at 2x (487ns/1024 cols)
            # vs ScalarE's 1x (1148ns), so the split is 1408/640 to balance
            # both at ~0.7us; the result ships as 3 pieces on 3 DMA queues.
            o = opool.tile([P, F], f16, tag="ot", name=f"o{m}")
            nc.vector.tensor_scalar(o[:, 640:F], z_t[m][:, 640:F], nt, 0.0,
                                    op0=AO.add, op1=AO.max)
            nc.gpsimd.dma_start(out=out[m * P:(m + 1) * P, 1344:F],
                                in_=o[:, 1344:F])
            nc.sync.dma_start(out=out[m * P:(m + 1) * P, 640:1344],
                              in_=o[:, 640:1344])
            nc.scalar.activation(o[:, 0:640], z_t[m][:, 0:640], relu,
                                 bias=nt, scale=1.0)
            nc.scalar.dma_start(out=out[m * P:(m + 1) * P, 0:640],
                                in_=o[:, 0:640])

        # ---- phase A: tiles 0-3 x cp0, K-major wavefront ----
        psA = [psum.tile([P, PW], f32, tag="ps", name=f"psA{m}")
               for m in range(4)]
        for k in range(KC):
            last = k == KC - 1
            for m in range(4):
                for h in range(2):
                    hs = slice(h * 512, (h + 1) * 512)
                    nc.tensor.matmul(
                        psA[m][:, hs], x_ap(k, m), w0_t[k][:, hs],
                        start=(k == 0), stop=last)
                if last:
                    drain_and_max(m, 0, psA[m])

        # ---- phase B: tiles 4-7 x cp0, tile-major ----
        for m in range(4, MT):
            gemm_block(m, 0)
        # ---- phase C: tiles 0-3 x cp1, tile-major + tails ----
        for m in range(4):
            gemm_block(m, 1)
            tail(m)
        # ---- phase D: tiles 4-7 x cp1, tile-major + tails ----
        for m in range(4, MT):
            gemm_block(m, 1)
            tail(m)

    nc.compile()
    return nc


_PROGRAMS: dict = {}


def _get_program(with_prior: bool, nseg: int):
    key = (with_prior, nseg)
    if key not in _PROGRAMS:
        _PROGRAMS[key] = build_program(with_prior, nseg)
    return _PROGRAMS[key]


def _fold_host(W, b, gamma, beta, moving_mean, moving_var):
    inv = (gamma / np.sqrt(moving_var + np.float32(BN_EPS))).astype(np.float32)
    Wp16 = np.ascontiguousarray((W * inv[None, :]).astype(np.float16))
    bp = (beta + (b - moving_mean) * inv).astype(np.float32)
    return Wp16, bp


def _prep_x(inputs):
    # xs[core, k, p, m*128 + b] = x[core*1024 + m*128 + b, k*128 + p]
    xc = inputs.astype(np.float16).reshape(NCORES, MT, P, KC, P)
    return np.ascontiguousarray(
        xc.transpose(0, 3, 4, 1, 2).reshape(NCORES, KC, P, BS))


def _prep_w(Wp16):
    # wprep[cp, p, k, c] = Wf[k*128 + p, cp*1024 + c]
    wc = Wp16.reshape(KC, P, 2, PW)
    return np.ascontiguousarray(wc.transpose(2, 1, 0, 3))


def _rkg_const():
    ij = np.arange(9, dtype=np.float32)
    s = ij[:, None] + ij[None, :]
    s[0, 0] = 1.0
    g = (-1.0 / s).astype(np.float32)
    return np.ascontiguousarray(np.broadcast_to(g[None], (P, 9, 9)))


def _prep_b(bp, nseg):
    ncand = 8 * nseg
    rkv = -1.0 / np.arange(1, ncand + 1, dtype=np.float32)
    row = np.concatenate([bp, rkv]).astype(np.float32)
    return np.ascontiguousarray(np.broadcast_to(row[None, :], (P, F + ncand)))


def _run(with_prior: bool, niters_or_nseg, xprep, Wp16, bp_rep, prior=None,
         nseg: int = 2):
    nc = _get_program(with_prior, nseg)
    wrep = _prep_w(Wp16)
    in_maps = []
    for c in range(NCORES):
        m = {"xs": xprep[c], "wprep": wrep, "bprep": bp_rep}
        if nseg == 2:
            m["rkg"] = _rkg_const()
        if with_prior:
            m["prior"] = np.ascontiguousarray(prior[c * BS:(c + 1) * BS, :])
        in_maps.append(m)
    res = bass_utils.run_bass_kernel_spmd(nc, in_maps, core_ids=list(range(NCORES)))
    return np.concatenate([r["out"] for r in res.results], axis=0)


def kernel(inputs, W, b, gamma, beta, moving_mean, moving_var, prior_scales):
    inputs = np.ascontiguousarray(np.asarray(inputs, dtype=np.float32))
    W = np.ascontiguousarray(np.asarray(W, dtype=np.float32))
    b = np.asarray(b, dtype=np.float32)
    gamma = np.asarray(gamma, dtype=np.float32)
    beta = np.asarray(beta, dtype=np.float32)
    moving_mean = np.asarray(moving_mean, dtype=np.float32)
    moving_var = np.asarray(moving_var, dtype=np.float32)
    prior_scales = np.asarray(prior_scales, dtype=np.float32)

    Wp16, bp = _fold_host(W, b, gamma, beta, moving_mean, moving_var)
    xprep = _prep_x(inputs)

    # prior==1 exactly -> multiplying by it is an algebraic no-op; skip it.
    with_prior = not bool(np.all(prior_scales == np.float32(1.0)))

    out16 = _run(with_prior, DEFAULT_ITERS, xprep, Wp16, _prep_b(bp, 2),
                 prior_scales)
    out = out16.astype(np.float32)

    # sparsemax rows must sum to ~1; if support ever exceeds the 16-candidate
    # coverage (never observed for this data), rerun with 32 candidates.
    rs = out.sum(axis=1, dtype=np.float64)
    if not np.all(np.abs(rs - 1.0) < 0.1):
        out = _run(with_prior, DEFAULT_ITERS, xprep, Wp16, _prep_b(bp, 4),
                   prior_scales, nseg=4)
        out = out.astype(np.float32)
    return out
